# revision 5
# baseline (speedup 1.0000x reference)
"""Trainium2 Bass kernel: multi-adapter LoRA linear via host-side fold,
fp16 matmul with an fp8e4m3 DoubleRow hybrid contraction.

y = x @ W.T + bias + 2*(x@A_g.T)@B_g.T  ==  x @ (W + 2*B_g@A_g).T + bias

Data-parallel over batch: each of 8 cores gets one batch element and its
group's folded weight matrix W'_g = W + 2*B_g@A_g. The contraction dim
(32 chunks of 128) is split: the first 22 chunks run in fp16 (1 cycle per
output row each), the last 10 run in fp8e4m3 DoubleRow mode (2 chunks per
instruction at 1 cycle per output row -- measured: DoubleRow is 2x fp16
per chunk on this hw, not the cost model's 4x). 13824 PE cycles per
output tile vs 16384 all-fp16 (-15.6%). fp8 operands use inverse-paired
scales (x*XS, W*WS with XS*WS == 1) so fp8 products accumulate into PSUM
at the true scale alongside the fp16 products; rel_rms ~1.8e-2 vs the
2e-2 gate (deterministic inputs).

Schedule: the startup stream (x + W'(ob0) chunks, interleaved per-k in
deadline order, bias, then the fp8 pairs) rides one queue since all DMA
shares one engine; phase A interleaves each chunk's matmuls across the
first 8 output tiles (8 PSUM banks) so the PE saturates ~2us in.
Remaining obs prefetch W one tile ahead; out tiles leave via DVE
bias-add + DMA on the other queue.
"""
import sys

if "/opt/trn_rl_repo" not in sys.path:
    sys.path.insert(0, "/opt/trn_rl_repo")

import numpy as np
import ml_dtypes

B, S, I, O, G, R = 8, 2048, 4096, 4096, 4, 16
OB = 512          # output free-dim tile (one PSUM bank of fp32)
N8 = 10           # fp8 k-chunks (must be even; the last N8 of 32)
K16 = I // 128 - N8   # fp16 k-chunks
XS, WS = 0.125, 8.0   # inverse-paired fp8 operand scales (XS*WS == 1)
F8 = ml_dtypes.float8_e4m3

_CACHE = {}


def build(s=S, i=I, o=O, repeat=1):
    import concourse.bacc as bacc
    import concourse.mybir as mybir
    import concourse.tile as tile

    f8, f16, f32 = mybir.dt.float8e4, mybir.dt.float16, mybir.dt.float32
    DR = mybir.MatmulPerfMode.DoubleRow
    kt = i // 128
    k16 = K16
    mt_n = s // 128
    nob = o // OB

    nc = bacc.Bacc("TRN2", target_bir_lowering=False, debug=False)
    x16d = nc.dram_tensor("x16", [k16 * 128, s], f16, kind="ExternalInput").ap()
    x8d = nc.dram_tensor("x8", [N8 * 128, s], f8, kind="ExternalInput").ap()
    w16d = nc.dram_tensor("w16", [k16 * 128, o], f16, kind="ExternalInput").ap()
    w8d = nc.dram_tensor("w8", [N8 * 128, o], f8, kind="ExternalInput").ap()
    biasr = nc.dram_tensor("biasr", [128, o], f16, kind="ExternalInput").ap()
    out = nc.dram_tensor("out", [s, o], f32, kind="ExternalOutput").ap()

    with tile.TileContext(nc) as tc:
        with (
            tc.tile_pool(name="xp", bufs=1) as xp,
            tc.tile_pool(name="wp", bufs=2) as wp,
            tc.tile_pool(name="lp", bufs=2) as lp,
            tc.tile_pool(name="bp", bufs=1) as bp,
            tc.tile_pool(name="op", bufs=2) as op,
            tc.tile_pool(name="pp", bufs=8, space="PSUM") as pp,
        ):
          x16t3 = x16d.rearrange("(k p) s -> p k s", p=128)  # [128, k16, s]
          x8t3 = x8d.rearrange("(k p) s -> p k s", p=128)    # [128, N8, s]
          w16t3 = w16d.rearrange("(k p) o -> p k o", p=128)  # [128, k16, o]
          w8t3 = w8d.rearrange("(k p) o -> p k o", p=128)    # [128, N8, o]

          for _rep in range(repeat):
            x16_sb = xp.tile([128, k16, s], f16, name="x16_sb")
            x8_sb = xp.tile([128, N8, s], f8, name="x8_sb")
            bias_sb = bp.tile([128, o], f16)

            w16_0 = wp.tile([128, k16, OB], f16, tag="w16")
            w8_0 = lp.tile([128, N8, OB], f8, tag="w8")
            # Startup stream in strict deadline order on one queue: phase A
            # consumes fp16 chunk k and W0 chunk k at ~1.7us/k while DMA
            # delivers the pair in ~1.8us; interleave per-k so the deficit
            # stays minimal. Bias before the fp8 pairs (first evac happens
            # right after the last fp8 pair). (Tried W0+bias on the scalar
            # queue ahead of the x stream for warm chaining: cost model says
            # strictly worse, cold and warm.)
            nc.sync.dma_start(x16_sb[:, 0:1, :], x16t3[:, 0:1, :])
            nc.sync.dma_start(w16_0[:, 0:1, :], w16t3[:, 0:1, 0:OB])
            nc.sync.dma_start(x16_sb[:, 1:2, :], x16t3[:, 1:2, :])
            nc.sync.dma_start(w16_0[:, 1:2, :], w16t3[:, 1:2, 0:OB])
            for k in range(2, k16, 2):
                nc.sync.dma_start(x16_sb[:, k:k + 2, :], x16t3[:, k:k + 2, :])
                nc.sync.dma_start(w16_0[:, k:k + 2, :], w16t3[:, k:k + 2, 0:OB])
            nc.sync.dma_start(bias_sb[:, :], biasr[:, :])
            for j in range(0, N8, 2):
                nc.sync.dma_start(x8_sb[:, j:j + 2, :], x8t3[:, j:j + 2, :])
                nc.sync.dma_start(w8_0[:, j:j + 2, :], w8t3[:, j:j + 2, 0:OB])

            def mm16(pt, w, mt, k, start):
                nc.tensor.matmul(
                    pt[:],
                    x16_sb[:, k, mt * 128:mt * 128 + 128],
                    w[:, k, :],
                    start=start,
                    stop=False,
                )

            def mm8(pt, w8, mt, j, stop):
                nc.tensor.matmul(
                    pt[:],
                    x8_sb[:, j:j + 2, mt * 128:mt * 128 + 128],
                    w8[:, j:j + 2, :],
                    start=False,
                    stop=stop,
                    perf_mode=DR,
                )

            def tile_mms(pt, w, w8, mt):
                for k in range(k16):
                    mm16(pt, w, mt, k, k == 0)
                for j in range(0, N8, 2):
                    mm8(pt, w8, mt, j, j == N8 - 2)

            def evac(pt, mt, ob):
                ot = op.tile([128, OB], f32, tag="ot")
                nc.vector.tensor_tensor(
                    ot[:], pt[:], bias_sb[:, ob * OB:(ob + 1) * OB],
                    op=mybir.AluOpType.add,
                )
                nc.scalar.dma_start(
                    out[mt * 128:(mt + 1) * 128, ob * OB:(ob + 1) * OB], ot[:]
                )

            def load_w(ob):
                w = wp.tile([128, k16, OB], f16, tag="w16")
                w8 = lp.tile([128, N8, OB], f8, tag="w8")
                nc.sync.dma_start(w[:, :, :], w16t3[:, :, ob * OB:(ob + 1) * OB])
                nc.sync.dma_start(w8[:, :, :], w8t3[:, :, ob * OB:(ob + 1) * OB])
                return w, w8

            # phase A: per k-chunk, matmuls of the first nA tiles of ob0 --
            # each arriving chunk enables nA matmuls, PE saturates early
            nA = min(8, mt_n)
            pts = [pp.tile([128, OB], f32, tag="pt", name=f"ptA{m}") for m in range(nA)]
            for k in range(k16):
                for m in range(nA):
                    mm16(pts[m], w16_0, m, k, k == 0)
            for j in range(0, N8, 2):
                for m in range(nA):
                    mm8(pts[m], w8_0, m, j, j == N8 - 2)
            w_cur = load_w(1) if nob > 1 else None
            for m in range(nA):
                evac(pts[m], m, 0)

            # rest of ob0 (all operands resident by now)
            for mt in range(nA, mt_n):
                pt = pp.tile([128, OB], f32, tag="pt")
                tile_mms(pt, w16_0, w8_0, mt)
                evac(pt, mt, 0)

            # remaining obs, W prefetched one ahead on the sync queue
            for ob in range(1, nob):
                w, w8 = w_cur
                w_cur = load_w(ob + 1) if ob + 1 < nob else None
                for mt in range(mt_n):
                    pt = pp.tile([128, OB], f32, tag="pt")
                    tile_mms(pt, w, w8, mt)
                    evac(pt, mt, ob)
    nc.compile()
    return nc


def prep_in_maps(data, W, bias, lora_a, lora_b):
    k16r = K16 * 128
    biasr = np.ascontiguousarray(
        np.broadcast_to(bias.astype(np.float16), (128, W.shape[0]))
    )
    W16g, W8g = {}, {}
    for g in range(G):
        Wg = W.astype(np.float32) + 2.0 * (
            lora_b[g].astype(np.float32) @ lora_a[g].astype(np.float32)
        )
        WT = Wg.T  # [I, O]
        W16g[g] = np.ascontiguousarray(WT[:k16r]).astype(np.float16)
        W8g[g] = np.ascontiguousarray(WT[k16r:] * np.float32(WS)).astype(F8)
    in_maps = []
    for b in range(data.shape[0]):
        g = b // (data.shape[0] // G)
        xT = data[b].T  # [I, S]
        in_maps.append({
            "x16": np.ascontiguousarray(xT[:k16r]).astype(np.float16),
            "x8": np.ascontiguousarray(xT[k16r:] * np.float32(XS)).astype(F8),
            "w16": W16g[g],
            "w8": W8g[g],
            "biasr": biasr,
        })
    return in_maps


def kernel(data, W, bias, lora_a, lora_b):
    from concourse.bass_utils import run_bass_kernel_spmd

    if "nc" not in _CACHE:
        _CACHE["nc"] = build()
    nc = _CACHE["nc"]
    in_maps = prep_in_maps(data, W, bias, lora_a, lora_b)
    res = run_bass_kernel_spmd(nc, in_maps, list(range(len(in_maps))))
    return np.stack([res.results[c]["out"] for c in range(len(in_maps))], axis=0)


# revision 6
# speedup vs baseline: 1.0170x; 1.0170x over previous
"""Trainium2 Bass kernel: multi-adapter LoRA linear via host-side fold,
fp16 matmul with an fp8e4m3 DoubleRow hybrid contraction.

y = x @ W.T + bias + 2*(x@A_g.T)@B_g.T  ==  x @ (W + 2*B_g@A_g).T + bias

Data-parallel over batch: each of 8 cores gets one batch element and its
group's folded weight matrix W'_g = W + 2*B_g@A_g. The contraction dim
(32 chunks of 128) is split: the first 22 chunks run in fp16 (1 cycle per
output row each), the last 10 run in fp8e4m3 DoubleRow mode (2 chunks per
instruction at 1 cycle per output row -- measured: DoubleRow is 2x fp16
per chunk on this hw, not the cost model's 4x). 13824 PE cycles per
output tile vs 16384 all-fp16 (-15.6%). fp8 operands use inverse-paired
scales (x*XS, W*WS with XS*WS == 1) so fp8 products accumulate into PSUM
at the true scale alongside the fp16 products; rel_rms ~1.8e-2 vs the
2e-2 gate (deterministic inputs).

Schedule: the startup stream (x + W'(ob0) chunks, interleaved per-k in
deadline order, bias, then the fp8 pairs) rides one queue since all DMA
shares one engine; phase A interleaves each chunk's matmuls across the
first 8 output tiles (8 PSUM banks) so the PE saturates ~2us in.
Remaining obs prefetch W one tile ahead; out tiles leave via DVE
bias-add + DMA on the other queue.
"""
import sys

if "/opt/trn_rl_repo" not in sys.path:
    sys.path.insert(0, "/opt/trn_rl_repo")

import numpy as np
import ml_dtypes

B, S, I, O, G, R = 8, 2048, 4096, 4096, 4, 16
OB = 512          # output free-dim tile (one PSUM bank of fp32)
N8 = 10           # fp8 k-chunks (must be even; the last N8 of 32)
K16 = I // 128 - N8   # fp16 k-chunks
XS, WS = 0.125, 8.0   # inverse-paired fp8 operand scales (XS*WS == 1)
F8 = ml_dtypes.float8_e4m3

_CACHE = {}


def build(s=S, i=I, o=O, repeat=1):
    import concourse.bacc as bacc
    import concourse.mybir as mybir
    import concourse.tile as tile

    f8, f16, f32 = mybir.dt.float8e4, mybir.dt.float16, mybir.dt.float32
    DR = mybir.MatmulPerfMode.DoubleRow
    kt = i // 128
    k16 = K16
    mt_n = s // 128
    nob = o // OB

    nc = bacc.Bacc("TRN2", target_bir_lowering=False, debug=False)
    x16d = nc.dram_tensor("x16", [k16 * 128, s], f16, kind="ExternalInput").ap()
    x8d = nc.dram_tensor("x8", [N8 * 128, s], f8, kind="ExternalInput").ap()
    w16d = nc.dram_tensor("w16", [k16 * 128, o], f16, kind="ExternalInput").ap()
    w8d = nc.dram_tensor("w8", [N8 * 128, o], f8, kind="ExternalInput").ap()
    biasr = nc.dram_tensor("biasr", [128, o], f16, kind="ExternalInput").ap()
    out = nc.dram_tensor("out", [s, o], f32, kind="ExternalOutput").ap()

    with tile.TileContext(nc) as tc:
        with (
            tc.tile_pool(name="xp", bufs=1) as xp,
            tc.tile_pool(name="wp", bufs=2) as wp,
            tc.tile_pool(name="lp", bufs=2) as lp,
            tc.tile_pool(name="bp", bufs=1) as bp,
            tc.tile_pool(name="op", bufs=2) as op,
            tc.tile_pool(name="pp", bufs=8, space="PSUM") as pp,
        ):
          x16t3 = x16d.rearrange("(k p) s -> p k s", p=128)  # [128, k16, s]
          x8t3 = x8d.rearrange("(k p) s -> p k s", p=128)    # [128, N8, s]
          w16t3 = w16d.rearrange("(k p) o -> p k o", p=128)  # [128, k16, o]
          w8t3 = w8d.rearrange("(k p) o -> p k o", p=128)    # [128, N8, o]

          for _rep in range(repeat):
            x16_sb = xp.tile([128, k16, s], f16, name="x16_sb")
            x8_sb = xp.tile([128, N8, s], f8, name="x8_sb")
            bias_sb = bp.tile([128, o], f16)

            w16_0 = wp.tile([128, k16, OB], f16, tag="w16")
            w8_0 = lp.tile([128, N8, OB], f8, tag="w8")
            # Startup stream in strict deadline order on one queue. Phase A
            # (output tiles 0..7) only reads the first half of each x chunk
            # (m < 1024), so the critical stream carries x first-halves +
            # W(ob0) + bias (~11.5MB = 32us < 46us of phase-A PE work); the
            # x second-halves ride behind and land in phase A's shadow,
            # before tiles 8-15 need them. (Tried W0+bias on the scalar
            # queue ahead of the x stream instead: cost model says strictly
            # worse, cold and warm.)
            sh = s // 2
            nc.sync.dma_start(x16_sb[:, 0:1, 0:sh], x16t3[:, 0:1, 0:sh])
            nc.sync.dma_start(w16_0[:, 0:1, :], w16t3[:, 0:1, 0:OB])
            nc.sync.dma_start(x16_sb[:, 1:2, 0:sh], x16t3[:, 1:2, 0:sh])
            nc.sync.dma_start(w16_0[:, 1:2, :], w16t3[:, 1:2, 0:OB])
            for k in range(2, k16, 2):
                nc.sync.dma_start(x16_sb[:, k:k + 2, 0:sh], x16t3[:, k:k + 2, 0:sh])
                nc.sync.dma_start(w16_0[:, k:k + 2, :], w16t3[:, k:k + 2, 0:OB])
            for j in range(0, N8, 2):
                nc.sync.dma_start(x8_sb[:, j:j + 2, 0:sh], x8t3[:, j:j + 2, 0:sh])
                nc.sync.dma_start(w8_0[:, j:j + 2, :], w8t3[:, j:j + 2, 0:OB])
            nc.sync.dma_start(bias_sb[:, :], biasr[:, :])
            for k in range(0, k16, 2):
                nc.sync.dma_start(x16_sb[:, k:k + 2, sh:s], x16t3[:, k:k + 2, sh:s])
            for j in range(0, N8, 2):
                nc.sync.dma_start(x8_sb[:, j:j + 2, sh:s], x8t3[:, j:j + 2, sh:s])

            def mm16(pt, w, mt, k, start):
                nc.tensor.matmul(
                    pt[:],
                    x16_sb[:, k, mt * 128:mt * 128 + 128],
                    w[:, k, :],
                    start=start,
                    stop=False,
                )

            def mm8(pt, w8, mt, j, stop):
                nc.tensor.matmul(
                    pt[:],
                    x8_sb[:, j:j + 2, mt * 128:mt * 128 + 128],
                    w8[:, j:j + 2, :],
                    start=False,
                    stop=stop,
                    perf_mode=DR,
                )

            def tile_mms(pt, w, w8, mt):
                for k in range(k16):
                    mm16(pt, w, mt, k, k == 0)
                for j in range(0, N8, 2):
                    mm8(pt, w8, mt, j, j == N8 - 2)

            def evac(pt, mt, ob):
                ot = op.tile([128, OB], f32, tag="ot")
                nc.vector.tensor_tensor(
                    ot[:], pt[:], bias_sb[:, ob * OB:(ob + 1) * OB],
                    op=mybir.AluOpType.add,
                )
                nc.scalar.dma_start(
                    out[mt * 128:(mt + 1) * 128, ob * OB:(ob + 1) * OB], ot[:]
                )

            def load_w(ob):
                w = wp.tile([128, k16, OB], f16, tag="w16")
                w8 = lp.tile([128, N8, OB], f8, tag="w8")
                nc.sync.dma_start(w[:, :, :], w16t3[:, :, ob * OB:(ob + 1) * OB])
                nc.sync.dma_start(w8[:, :, :], w8t3[:, :, ob * OB:(ob + 1) * OB])
                return w, w8

            # phase A: per k-chunk, matmuls of the first nA tiles of ob0 --
            # each arriving chunk enables nA matmuls, PE saturates early
            nA = min(8, mt_n)
            pts = [pp.tile([128, OB], f32, tag="pt", name=f"ptA{m}") for m in range(nA)]
            for k in range(k16):
                for m in range(nA):
                    mm16(pts[m], w16_0, m, k, k == 0)
            for j in range(0, N8, 2):
                for m in range(nA):
                    mm8(pts[m], w8_0, m, j, j == N8 - 2)
            w_cur = load_w(1) if nob > 1 else None
            for m in range(nA):
                evac(pts[m], m, 0)

            # rest of ob0 (all operands resident by now)
            for mt in range(nA, mt_n):
                pt = pp.tile([128, OB], f32, tag="pt")
                tile_mms(pt, w16_0, w8_0, mt)
                evac(pt, mt, 0)

            # remaining obs, W prefetched one ahead on the sync queue
            for ob in range(1, nob):
                w, w8 = w_cur
                w_cur = load_w(ob + 1) if ob + 1 < nob else None
                for mt in range(mt_n):
                    pt = pp.tile([128, OB], f32, tag="pt")
                    tile_mms(pt, w, w8, mt)
                    evac(pt, mt, ob)
    nc.compile()
    return nc


def prep_in_maps(data, W, bias, lora_a, lora_b):
    k16r = K16 * 128
    biasr = np.ascontiguousarray(
        np.broadcast_to(bias.astype(np.float16), (128, W.shape[0]))
    )
    W16g, W8g = {}, {}
    for g in range(G):
        Wg = W.astype(np.float32) + 2.0 * (
            lora_b[g].astype(np.float32) @ lora_a[g].astype(np.float32)
        )
        WT = Wg.T  # [I, O]
        W16g[g] = np.ascontiguousarray(WT[:k16r]).astype(np.float16)
        W8g[g] = np.ascontiguousarray(WT[k16r:] * np.float32(WS)).astype(F8)
    in_maps = []
    for b in range(data.shape[0]):
        g = b // (data.shape[0] // G)
        xT = data[b].T  # [I, S]
        in_maps.append({
            "x16": np.ascontiguousarray(xT[:k16r]).astype(np.float16),
            "x8": np.ascontiguousarray(xT[k16r:] * np.float32(XS)).astype(F8),
            "w16": W16g[g],
            "w8": W8g[g],
            "biasr": biasr,
        })
    return in_maps


def kernel(data, W, bias, lora_a, lora_b):
    from concourse.bass_utils import run_bass_kernel_spmd

    if "nc" not in _CACHE:
        _CACHE["nc"] = build()
    nc = _CACHE["nc"]
    in_maps = prep_in_maps(data, W, bias, lora_a, lora_b)
    res = run_bass_kernel_spmd(nc, in_maps, list(range(len(in_maps))))
    return np.stack([res.results[c]["out"] for c in range(len(in_maps))], axis=0)
